# revision 3
# baseline (speedup 1.0000x reference)
"""Trainium2 Bass kernel for nn_AttentionBlock (dense_transformer).

Reference computation (per batch b):
    qs[t,j]    = sum_i s[t,i] Q[h,i,j]
    Omega[t,u] = sum_j qs[t,j] s[u,j]       (masked causal: keep u <= t)
    es[u,i]    = sum_j E[h,i,j] s[u,j]
    r[t,i]     = sum_h sum_u Omega[t,u] es[u,i]

Distribution: data-parallel over batch b — 8 batches, 8 NeuronCores, one
batch per core, no collectives. Q/E replicated.

Per-core kernel works entirely in "transposed" space to keep every matmul
contraction on the partition axis with zero on-chip transposes:
    sT[i,t]      (host-pretransposed s[b].T)
    qsT[j,t]     = matmul(lhsT=Q[h][i-chunk, j-chunk], rhs=sT[i-chunk, :])
    es[u,i]      = matmul(lhsT=sT[j-chunk, u-block],   rhs=ET[h][j-chunk, :])
    OmegaT[u,t]  = matmul(lhsT=sT[j-chunk, u-block],   rhs=qsT[j-chunk, t-chunk])
                   -- computed only for causal blocks t >= u-block; diagonal
                      128x128 block masked elementwise (keep u <= t)
    r[t,i]      += matmul(lhsT=OmegaT[u-block, t-block], rhs=es[u-block, :])
                   -- PSUM-accumulated over u-blocks, SBUF-accumulated over h

All matmuls run as float32r (fp32 storage, full 1 cycle/row TensorE rate at
free-dim >= 256).
"""

import numpy as np

import concourse.bacc as bacc
import concourse.mybir as mybir
import concourse.tile as tile
from concourse.bass_utils import run_bass_kernel_spmd
from concourse.masks import make_upper_triangular

B = 8      # batch (== number of cores)
T = 1024   # tokens
NF = 256   # feature dim n
H = 8      # heads
P = 128    # partitions
TB = T // P    # 8 token blocks
JC = NF // P   # 2 feature chunks
NCORES = 8

F32 = mybir.dt.float32
F32R = mybir.dt.float32r


def _om_chunks(width):
    """Split `width` (>=256) into free-dim chunks <=512, each >=256."""
    chunks = []
    while width > 512:
        if width - 512 >= 256:
            chunks.append(512)
            width -= 512
        else:
            chunks.append(width - 256)
            width = 256
    chunks.append(width)
    return chunks


def _emit(tc, nc, sT_d, Q_d, ET_d, out_d, ctx):
    res = ctx.enter_context(tc.tile_pool(name="res", bufs=1))
    work = ctx.enter_context(tc.tile_pool(name="work", bufs=2))
    psq = ctx.enter_context(tc.tile_pool(name="psq", bufs=2, space="PSUM"))
    pse = ctx.enter_context(tc.tile_pool(name="pse", bufs=2, space="PSUM"))
    pso = ctx.enter_context(tc.tile_pool(name="pso", bufs=2, space="PSUM"))
    psr = ctx.enter_context(tc.tile_pool(name="psr", bufs=2, space="PSUM"))

    sT = res.tile([P, JC, T], F32R)        # [p=i%128, i//128, t]
    Qs = res.tile([P, H * JC, NF], F32R)   # [p=i%128, h*2+(i//128), j]
    ETs = res.tile([P, H * JC, NF], F32R)  # [p=j%128, h*2+(j//128), i]
    mask = res.tile([P, P], F32)          # mask[u,t] = 1 if u <= t else 0
    r_sb = res.tile([P, TB, NF], F32)     # [p=t%128, t//128, i]

    make_upper_triangular(nc, mask, val=1.0, diag=True)

    for ic in range(JC):
        nc.sync.dma_start(out=sT[:, ic, :], in_=sT_d[ic * P:(ic + 1) * P, :].bitcast(F32R))
    for h in range(H):
        for c in range(JC):
            nc.sync.dma_start(
                out=Qs[:, h * JC + c, :], in_=Q_d[h, c * P:(c + 1) * P, :].bitcast(F32R))
            nc.sync.dma_start(
                out=ETs[:, h * JC + c, :], in_=ET_d[h, c * P:(c + 1) * P, :].bitcast(F32R))

    for h in range(H):
        # qsT[j,t] = sum_i Q[h,i,j] sT[i,t]
        qsT = work.tile([P, JC, T], F32R, tag="qsT")
        for jc in range(JC):
            for tc_i in range(T // 512):
                pq = psq.tile([P, 512], F32, tag="pq")
                for ic in range(JC):
                    nc.tensor.matmul(
                        pq,
                        lhsT=Qs[:, h * JC + ic, jc * P:(jc + 1) * P],
                        rhs=sT[:, ic, tc_i * 512:(tc_i + 1) * 512],
                        start=(ic == 0),
                        stop=(ic == JC - 1),
                    )
                nc.vector.tensor_copy(
                    out=qsT[:, jc, tc_i * 512:(tc_i + 1) * 512], in_=pq)

        # es[u,i] = sum_j sT[j,u] ET[h,j,i]
        es = work.tile([P, TB, NF], F32R, tag="es")
        for uc in range(TB):
            pe = pse.tile([P, NF], F32, tag="pe")
            for jc in range(JC):
                nc.tensor.matmul(
                    pe,
                    lhsT=sT[:, jc, uc * P:(uc + 1) * P],
                    rhs=ETs[:, h * JC + jc, :],
                    start=(jc == 0),
                    stop=(jc == JC - 1),
                )
            nc.vector.tensor_copy(out=es[:, uc, :], in_=pe)

        # OmegaT[u,t] = sum_j sT[j,u] qsT[j,t], causal blocks only (t >= u)
        om = work.tile([P, TB, T], F32R, tag="om")
        for uc in range(TB):
            t_lo = min(uc * P, T - 256)  # pad last blocks to free-dim >= 256
            d_lo = uc * P                # diagonal block start
            cs = t_lo
            for cw in _om_chunks(T - t_lo):
                po = pso.tile([P, 512], F32, tag="po")
                for jc in range(JC):
                    nc.tensor.matmul(
                        po[:, :cw],
                        lhsT=sT[:, jc, uc * P:(uc + 1) * P],
                        rhs=qsT[:, jc, cs:cs + cw],
                        start=(jc == 0),
                        stop=(jc == JC - 1),
                    )
                if cs <= d_lo < cs + cw:
                    # chunk holds the diagonal block: mask it; anything
                    # before d_lo is fully-masked garbage (never read)
                    nc.vector.tensor_mul(
                        om[:, uc, d_lo:d_lo + P],
                        po[:, d_lo - cs:d_lo - cs + P],
                        mask,
                    )
                    if d_lo + P < cs + cw:
                        nc.vector.tensor_copy(
                            out=om[:, uc, d_lo + P:cs + cw],
                            in_=po[:, d_lo - cs + P:cw],
                        )
                else:
                    nc.vector.tensor_copy(
                        out=om[:, uc, cs:cs + cw], in_=po[:, :cw])
                cs += cw

        # r[t,i] += sum_{uc<=bt} OmegaT[uc][:, bt-block].T @ es[uc]
        for bt in range(TB):
            pr = psr.tile([P, NF], F32, tag="pr")
            for uc in range(bt + 1):
                nc.tensor.matmul(
                    pr,
                    lhsT=om[:, uc, bt * P:(bt + 1) * P],
                    rhs=es[:, uc, :],
                    start=(uc == 0),
                    stop=(uc == bt),
                )
            if h == 0:
                nc.vector.tensor_copy(out=r_sb[:, bt, :], in_=pr)
            else:
                nc.vector.tensor_add(r_sb[:, bt, :], r_sb[:, bt, :], pr)

    for bt in range(TB):
        nc.sync.dma_start(
            out=out_d[bt * P:(bt + 1) * P, :], in_=r_sb[:, bt, :])


def build():
    from contextlib import ExitStack

    nc = bacc.Bacc(
        "TRN2",
        target_bir_lowering=False,
        debug=False,
        enable_asserts=False,
        num_devices=NCORES,
    )
    sT_d = nc.dram_tensor("sT", [NF, T], F32, kind="ExternalInput").ap()
    Q_d = nc.dram_tensor("Q", [H, NF, NF], F32, kind="ExternalInput").ap()
    ET_d = nc.dram_tensor("ET", [H, NF, NF], F32, kind="ExternalInput").ap()
    out_d = nc.dram_tensor("out", [T, NF], F32, kind="ExternalOutput").ap()
    with tile.TileContext(nc) as tc:
        with ExitStack() as ctx:
            _emit(tc, nc, sT_d, Q_d, ET_d, out_d, ctx)
    nc.compile()
    return nc


_NC = None


def _get_nc():
    global _NC
    if _NC is None:
        _NC = build()
    return _NC


def _in_maps(s, Q, E):
    s = np.asarray(s, dtype=np.float32)
    Q = np.ascontiguousarray(np.asarray(Q, dtype=np.float32))
    ET = np.ascontiguousarray(
        np.asarray(E, dtype=np.float32).transpose(0, 2, 1))
    return [
        {"sT": np.ascontiguousarray(s[b].T), "Q": Q, "ET": ET}
        for b in range(B)
    ]


def kernel(s, Q, E):
    nc = _get_nc()
    res = run_bass_kernel_spmd(
        nc, _in_maps(s, Q, E), core_ids=list(range(NCORES)))
    return np.stack([res.results[b]["out"] for b in range(B)], axis=0)


def run_profiled(s, Q, E, tmpdir=None):
    """Like kernel() but with NTFF profiling; returns (out, exec_time_ns)."""
    nc = _get_nc()
    res = run_bass_kernel_spmd(
        nc, _in_maps(s, Q, E), core_ids=list(range(NCORES)),
        trace=True, tmpdir=tmpdir)
    out = np.stack([res.results[b]["out"] for b in range(B)], axis=0)
    return out, res.exec_time_ns


# revision 4
# speedup vs baseline: 1.0144x; 1.0144x over previous
"""Trainium2 Bass kernel (v9) for nn_AttentionBlock — reassociated causal attention.

Reference (per batch b):
    qs[t,j]    = sum_i s[t,i] Q[h,i,j]
    Omega[t,u] = sum_j qs[t,j] s[u,j]       (causal: keep u <= t)
    es[u,i]    = sum_j E[h,i,j] s[u,j]
    r[t,i]     = sum_h sum_u Omega[t,u] es[u,i]

Reassociation: for full (below-diagonal) 128-token blocks,
    sum_{u in blk} Omega[t,u] es[u,i] = qs[t,:] @ (s[blk].T @ es[blk])
so r's off-diagonal part = qs[bt] @ Gsum(bt) with Gsum the PSUM-accumulated
prefix of G_uc = s[uc].T @ es[uc]; only diagonal 128x128 Omega blocks are
materialized, masked by a DVE multiply with a precomputed triangular mask.

v3 over v2: consolidated input DMAs split across sync+scalar HWDGE queues
(cuts the serialized descriptor-issue ramp), next head's qsT/es matmul groups
software-pipelined into the current head's t-block loop (fills PE stalls on
the Gsum-snapshot chain), diag mask fused into one DVE tensor_mul (drops the
ACT-copy -> GpSimd-select chain), parallel final drain.

Distribution: data-parallel over batch (8 batches = 8 cores, no collectives).
All matmuls bf16; f32 PSUM accumulation; r lives in PSUM the whole kernel
(zeroed once, then start=False accumulate-or-overwrite via has_written).
"""

import numpy as np
import ml_dtypes

import concourse.bacc as bacc
import concourse.mybir as mybir
import concourse.tile as tile
from concourse.bass_utils import run_bass_kernel_spmd

B = 8      # batch (== number of cores)
T = 1024   # tokens
NF = 256   # feature dim n
H = 8      # heads
P = 128    # partitions
TB = T // P    # 8 token blocks
JC = NF // P   # 2 feature chunks
NCORES = 8

F32 = mybir.dt.float32
BF16 = mybir.dt.bfloat16
IS_GE = mybir.AluOpType.is_ge


def _emit(tc, nc, s_d, sT_d, Q_d, ET_d, out_d, ctx):
    res = ctx.enter_context(tc.tile_pool(name="res", bufs=1))
    work = ctx.enter_context(tc.tile_pool(name="work", bufs=2))
    snap = ctx.enter_context(tc.tile_pool(name="snap", bufs=3))
    prp = ctx.enter_context(tc.tile_pool(name="prp", bufs=1, space="PSUM"))
    pgp = ctx.enter_context(tc.tile_pool(name="pgp", bufs=1, space="PSUM"))
    pwp = ctx.enter_context(tc.tile_pool(name="pwp", bufs=3, space="PSUM"))

    s_sb = res.tile([P, TB, NF], BF16)      # [u%128, uc, j]
    sT_sb = res.tile([P, JC, T], BF16)      # [j%128, jc, t]
    Q_sb = res.tile([P, H * JC, NF], BF16)  # [i%128, h*2+ic, j]
    ET_sb = res.tile([P, H * JC, NF], BF16)  # [j%128, h*2+jc, i]
    mask = res.tile([P, 2, P], BF16)        # [u, pair, t]: 1 where u <= t
    r_out = res.tile([P, TB, NF], F32)

    # r accumulates here across the whole kernel: zero once, then every
    # matmul uses start=False (accumulate-or-overwrite via has_written).
    pr = prp.tile([P, TB, NF], F32)
    nc.vector.memset(pr, 0.0)

    nc.gpsimd.memset(mask, 1.0)
    nc.gpsimd.affine_select(
        out=mask, in_=mask,
        pattern=[[0, 2], [1, P]],
        compare_op=IS_GE,   # keep 1.0 where t - u >= 0, else 0
        fill=0.0, base=0, channel_multiplier=-1,
    )

    # Input DMAs: few big transfers, first-needed first, split across the
    # sync and scalar HWDGE queues so descriptor issue isn't serialized.
    nc.sync.dma_start(
        out=Q_sb[:, 0:JC, :],
        in_=Q_d[0].rearrange("(c p) j -> p c j", p=P))
    nc.sync.dma_start(
        out=sT_sb[:, :, 0:512],
        in_=sT_d[:, 0:512].rearrange("(c p) t -> p c t", p=P))
    nc.sync.dma_start(
        out=sT_sb[:, :, 512:],
        in_=sT_d[:, 512:].rearrange("(c p) t -> p c t", p=P))
    nc.sync.dma_start(
        out=Q_sb[:, JC:2 * JC, :],
        in_=Q_d[1].rearrange("(c p) j -> p c j", p=P))
    nc.sync.dma_start(
        out=s_sb, in_=s_d.rearrange("(c p) j -> p c j", p=P))
    nc.sync.dma_start(
        out=Q_sb[:, 2 * JC:4 * JC, :],
        in_=Q_d[2:4].rearrange("h (c p) j -> p (h c) j", p=P))
    nc.sync.dma_start(
        out=Q_sb[:, 4 * JC:, :],
        in_=Q_d[4:].rearrange("h (c p) j -> p (h c) j", p=P))
    nc.gpsimd.dma_start(
        out=ET_sb[:, 0:JC, :],
        in_=ET_d[0].rearrange("(c p) j -> p c j", p=P))
    nc.gpsimd.dma_start(
        out=ET_sb[:, JC:4 * JC, :],
        in_=ET_d[1:4].rearrange("h (c p) j -> p (h c) j", p=P))
    nc.gpsimd.dma_start(
        out=ET_sb[:, 4 * JC:, :],
        in_=ET_d[4:].rearrange("h (c p) j -> p (h c) j", p=P))

    movers = [nc.vector.tensor_copy, nc.scalar.copy]
    mv = [0]

    def mover(out, in_):
        movers[mv[0] % 2](out=out, in_=in_)
        mv[0] += 1

    # ---- per-head prep: qsT and es, emitted as 8 independent PSUM groups
    def prep_groups(h):
        """Yields 8 emit-thunks: 4 qsT groups then 4 es pair-groups."""
        qsT = work.tile([P, JC, T], BF16, tag="qsT", name=f"qsT{h}")
        es = work.tile([P, TB, NF], BF16, tag="es", name=f"es{h}")

        def qsT_group(jc, tcx):
            pw = pwp.tile([P, 512], F32, tag="pw", name="pwq")
            for ic in range(JC):
                nc.tensor.matmul(
                    pw,
                    lhsT=Q_sb[:, h * JC + ic, jc * P:(jc + 1) * P],
                    rhs=sT_sb[:, ic, tcx * 512:(tcx + 1) * 512],
                    start=(ic == 0),
                    stop=(ic == JC - 1),
                )
            mover(qsT[:, jc, tcx * 512:(tcx + 1) * 512], pw)

        def es_group(up):
            pw = pwp.tile([P, 512], F32, tag="pw", name="pwe")
            for half in range(2):
                uc = 2 * up + half
                for jc in range(JC):
                    nc.tensor.matmul(
                        pw[:, half * NF:(half + 1) * NF],
                        lhsT=sT_sb[:, jc, uc * P:(uc + 1) * P],
                        rhs=ET_sb[:, h * JC + jc, :],
                        start=(half == 0 and jc == 0),
                        stop=(half == 1 and jc == JC - 1),
                        skip_group_check=True,
                    )
            mover(es[:, 2 * up:2 * up + 2, :], pw)

        thunks = []
        for jc in range(JC):
            for tcx in range(T // 512):
                thunks.append(lambda jc=jc, tcx=tcx: qsT_group(jc, tcx))
        for up in range(TB // 2):
            thunks.append(lambda up=up: es_group(up))
        return qsT, es, thunks

    def body(h, qsT, es, next_thunks, drain=False):
        """bt-loop for head h, with next head's prep groups interleaved."""
        pg = pgp.tile([P, JC, NF], F32, tag="pg")
        pwd = None
        omd = None
        for bt in range(TB):
            # diagonal OmegaT block [u,t] = sum_j sT[j,u] qsT[j,t]; two bt
            # share a bank; masked via one DVE multiply (keep u <= t)
            if bt % 2 == 0:
                pwd = pwp.tile([P, 512], F32, tag="pw", name="pwd")
                omd = snap.tile([P, 2, P], BF16, tag="omd")
            for jc in range(JC):
                nc.tensor.matmul(
                    pwd[:, (bt % 2) * P:(bt % 2 + 1) * P],
                    lhsT=sT_sb[:, jc, bt * P:(bt + 1) * P],
                    rhs=qsT[:, jc, bt * P:(bt + 1) * P],
                    start=(bt % 2 == 0 and jc == 0),
                    stop=(bt % 2 == 1 and jc == JC - 1),
                    skip_group_check=True,
                )
            # full part first: Gsum(bt) = sum_{uc<bt} s[uc].T @ es[uc]
            # (PSUM prefix accumulation); its PE matmuls run while the DVE
            # mask-multiply of the diag block completes
            gs = None
            if bt >= 1:
                uc = bt - 1
                for jc in range(JC):
                    nc.tensor.matmul(
                        pg[:, jc, :],
                        lhsT=s_sb[:, uc, jc * P:(jc + 1) * P],
                        rhs=es[:, uc, :],
                        start=(bt == 1 and jc == 0),
                        stop=(bt == TB - 1 and jc == JC - 1),
                        skip_group_check=True,
                    )
                if drain:
                    # bare chain: halve snapshot latency via both engines
                    gs = snap.tile([P, JC, NF], BF16, tag="gs")
                    nc.vector.tensor_copy(out=gs[:, 0, :], in_=pg[:, 0, :])
                    nc.scalar.copy(out=gs[:, 1, :], in_=pg[:, 1, :])
                else:
                    gs = snap.tile([P, JC, NF], BF16, tag="gs")
                    mover(gs, pg)
            if next_thunks:
                next_thunks.pop(0)()
            if bt % 2 == 1:
                nc.vector.tensor_mul(omd, pwd[:, 0:2 * P], mask)
                for b2 in (bt - 1, bt):
                    nc.tensor.matmul(
                        pr[:, b2, :],
                        lhsT=omd[:, b2 % 2, :],
                        rhs=es[:, b2, :],
                        start=False, stop=False, skip_group_check=True,
                    )
            if gs is not None:
                for jc in range(JC):
                    nc.tensor.matmul(
                        pr[:, bt, :],
                        lhsT=qsT[:, jc, bt * P:(bt + 1) * P],
                        rhs=gs[:, jc, :],
                        start=False, stop=False, skip_group_check=True,
                    )
            if drain and bt % 2 == 1:
                # h7: pr[bt-1], pr[bt] are final after this iteration
                mover(r_out[:, bt - 1:bt + 1, :], pr[:, bt - 1:bt + 1, :])
                if bt == 3:
                    nc.sync.dma_start(
                        out=out_d[0:T // 2, :].rearrange(
                            "(bt p) i -> p bt i", p=P),
                        in_=r_out[:, 0:TB // 2, :])
                elif bt == 5:
                    nc.scalar.dma_start(
                        out=out_d[T // 2:3 * T // 4, :].rearrange(
                            "(bt p) i -> p bt i", p=P),
                        in_=r_out[:, 4:6, :])
                elif bt == 7:
                    nc.sync.dma_start(
                        out=out_d[3 * T // 4:, :].rearrange(
                            "(bt p) i -> p bt i", p=P),
                        in_=r_out[:, 6:8, :])

    qsT, es, thunks = prep_groups(0)
    for th in thunks:
        th()
    for h in range(H):
        if h + 1 < H:
            nqsT, nes, nthunks = prep_groups(h + 1)
        else:
            nqsT, nes, nthunks = None, None, []
        body(h, qsT, es, nthunks, drain=(h == H - 1))
        for th in nthunks:   # any leftovers
            th()
        qsT, es = nqsT, nes



def build():
    from contextlib import ExitStack

    nc = bacc.Bacc(
        "TRN2",
        target_bir_lowering=False,
        debug=False,
        enable_asserts=False,
        num_devices=NCORES,
    )
    s_d = nc.dram_tensor("s", [T, NF], BF16, kind="ExternalInput").ap()
    sT_d = nc.dram_tensor("sT", [NF, T], BF16, kind="ExternalInput").ap()
    Q_d = nc.dram_tensor("Q", [H, NF, NF], BF16, kind="ExternalInput").ap()
    ET_d = nc.dram_tensor("ET", [H, NF, NF], BF16, kind="ExternalInput").ap()
    out_d = nc.dram_tensor("out", [T, NF], F32, kind="ExternalOutput").ap()
    with tile.TileContext(nc) as tc:
        with ExitStack() as ctx:
            _emit(tc, nc, s_d, sT_d, Q_d, ET_d, out_d, ctx)
    nc.compile()
    return nc


_NC = None


def _get_nc():
    global _NC
    if _NC is None:
        _NC = build()
    return _NC


def _in_maps(s, Q, E):
    bf = ml_dtypes.bfloat16
    s = np.asarray(s, dtype=np.float32)
    Qb = np.ascontiguousarray(np.asarray(Q, dtype=np.float32)).astype(bf)
    ETb = np.ascontiguousarray(
        np.asarray(E, dtype=np.float32).transpose(0, 2, 1)).astype(bf)
    return [
        {
            "s": np.ascontiguousarray(s[b]).astype(bf),
            "sT": np.ascontiguousarray(s[b].T).astype(bf),
            "Q": Qb,
            "ET": ETb,
        }
        for b in range(B)
    ]


def kernel(s, Q, E):
    nc = _get_nc()
    res = run_bass_kernel_spmd(
        nc, _in_maps(s, Q, E), core_ids=list(range(NCORES)))
    return np.stack([res.results[b]["out"] for b in range(B)], axis=0)


def run_profiled(s, Q, E, tmpdir=None):
    nc = _get_nc()
    res = run_bass_kernel_spmd(
        nc, _in_maps(s, Q, E), core_ids=list(range(NCORES)),
        trace=True, tmpdir=tmpdir)
    out = np.stack([res.results[b]["out"] for b in range(B)], axis=0)
    return out, res.exec_time_ns
